# revision 51
# baseline (speedup 1.0000x reference)
"""Causal self-attention (RoPE) Trainium2 kernel, 8-way sharded.

Sharding: core = (batch b in 0..1) x (head group g in 0..3, 4 heads each).
Each core computes its batch's attention for its 4 heads plus the partial
output projection; the host sums the 4 partials per batch.

v2: single fused per-chunk pipeline. Each 512-seq chunk does
QKV projection -> rope -> attention block (chunk i-block) -> per-block
normalization -> (next iter) output projection, so PE stays continuously
busy, DMA/ACT/DVE/Pool overlap, and output DMA streams per block.

Layout strategy (per core):
- host passes xT = x[b].T (fp16) so the embed dim lands on SBUF partitions.
- W_qkv columns are permuted so q^T/k^T emerge from the projection matmul
  already transposed, with RoPE even/odd dim pairs de-interleaved into
  x1/x2 partition blocks (scores are invariant to a head-dim permutation).
- all matmul operands are fp16; PSUM accumulation stays fp32.
- scores are computed transposed (sT[j,i]); softmax needs no max pass
  (|scores| < ~4) and the denominator is obtained by appending a ones
  column to V (M=65 PV matmuls). Normalization multiplies by exp(-ln Z)
  broadcast via a selector matmul, per i-block.
- causal masking: only j<=i column ranges are computed; the single
  diagonal 128x128 block per j-tile gets a triangle multiply.
- engine split: exp on ACT; rope-q/stash/casts on DVE; rope-k/triangle/
  norm-mult/casts on Pool; scores/PV/projections on PE.
"""
import sys

sys.path.insert(0, "/opt/trn_rl_repo")

import numpy as np

NUM_HEADS = 16
HEAD_DIM = 64
B, S, E = 2, 2048, 1024
HG = 4                      # heads per core
NG = NUM_HEADS // HG        # head groups
N_CORES = B * NG
F_QK = 2 * HG * HEAD_DIM    # 512 projected q+k rows per core
F_V = HG * HEAD_DIM         # 256 v cols per core
ESUB = E // 128             # 8 K-subtiles over embed dim
NCHUNK = 4                  # 512-col seq chunks (projection + i-blocks)
CHUNK = S // NCHUNK         # 512
NST = S // 128              # 16 seq tiles of 128
BLK = 512                   # attention i-block width == CHUNK
NBLK = S // BLK             # 4

_CACHE = {}


def _build_program():
    import concourse.bass as bass
    import concourse.mybir as mybir
    import concourse.tile as tile
    from concourse import bacc

    f32 = mybir.dt.float32
    f16 = mybir.dt.float16
    Alu = mybir.AluOpType
    Act = mybir.ActivationFunctionType

    # Force the single combined Ln+Exp activation-table set: hide Exp/Ln in
    # every other set so the table-load fixpoint can't alternate between
    # exp-only and ln-only sets (each mid-stream ACT_TABLE_LOAD costs ~1.3us
    # on the exp-critical scalar engine).
    from concourse.hw_specs import get_activation_tables as _gat
    if not getattr(bacc, "_one_set_patch", False):
        def _tables_one_set(arch, __orig=_gat):
            out = {}
            for name, funcs in __orig(arch).items():
                if name == "natural_log_exp_and_others":
                    out[name] = funcs
                else:
                    out[name] = {f for f in funcs
                                 if f not in (Act.Exp, Act.Ln)}
            return out
        bacc.get_activation_tables = _tables_one_set
        bacc._one_set_patch = True

    nc = bacc.Bacc("TRN2", target_bir_lowering=False, debug=False,
                   num_devices=N_CORES)

    xT_d = nc.dram_tensor("xT", (E, S), f16, kind="ExternalInput").ap()
    wqk_d = nc.dram_tensor("wqk", (E, F_QK), f16, kind="ExternalInput").ap()
    wv_d = nc.dram_tensor("wv", (E, F_V), f16, kind="ExternalInput").ap()
    wout_d = nc.dram_tensor("wout", (F_V, E), f16, kind="ExternalInput").ap()
    cs_d = nc.dram_tensor("cs", (128, S), f16, kind="ExternalInput").ap()
    sn_d = nc.dram_tensor("sn", (128, S), f16, kind="ExternalInput").ap()
    tri_d = nc.dram_tensor("tri", (128, 128), f16, kind="ExternalInput").ap()
    sel_d = nc.dram_tensor("sel", (2, 256), f16, kind="ExternalInput").ap()
    out_d = nc.dram_tensor("out", (S, E), f16, kind="ExternalOutput").ap()

    scale = 1.0 / float(np.sqrt(HEAD_DIM))

    with tile.TileContext(nc) as tc:
        with tc.tile_pool(name="per", bufs=1) as per, \
             tc.tile_pool(name="rsc", bufs=2) as rsc, \
             tc.tile_pool(name="pt", bufs=3) as ptp, \
             tc.tile_pool(name="ot", bufs=4) as otp, \
             tc.tile_pool(name="sm", bufs=2) as smp, \
             tc.tile_pool(name="pp", bufs=1, space="PSUM") as pp:
            # ---- persistent SBUF tiles ----
            xT_sb = per.tile([128, ESUB, S], f16)
            wqk_sb = per.tile([128, ESUB, F_QK], f16)
            wv_sb = per.tile([128, ESUB, F_V], f16)
            wout_sb = per.tile([128, 2, E], f16)
            cs_sb = per.tile([128, S], f16)
            sn_sb = per.tile([128, S], f16)
            tri_sb = per.tile([128, 128], f16)
            sel_sb = per.tile([2, 256], f16)
            qr = per.tile([128, 2, S], f16)    # rope out q: [:,0]=x1', [:,1]=x2'
            kr = per.tile([128, 2, S], f16)
            qp = per.tile([128, 2, S], f16)    # pair layout for scores
            kp = per.tile([128, 2, S], f16)
            v_sb = per.tile([128, NST, HG * 65], f16)
            ctxu_sb = per.tile([128, 2, S], f16)
            zall0 = per.tile([2, S], f16)
            zall1 = per.tile([2, S], f16)
            zalls = (zall0, zall1)

            # ---- input DMAs, split across the two HWDGE queues (sync=SP,
            # scalar=ACT) ordered by first use; first-needed tensors are
            # split in half so compute can start sooner ----
            xT_r = xT_d.rearrange("(o p) s -> p o s", p=128)
            wqk_r = wqk_d.rearrange("(o p) f -> p o f", p=128)
            c0 = slice(0, CHUNK)
            nc.sync.dma_start(xT_sb[:, 0:4, c0], xT_r[:, 0:4, c0])
            nc.scalar.dma_start(wqk_sb[:, 0:4, :], wqk_r[:, 0:4, :])
            nc.scalar.dma_start(wqk_sb[:, 4:8, :], wqk_r[:, 4:8, :])
            nc.scalar.dma_start(xT_sb[:, 4:8, c0], xT_r[:, 4:8, c0])
            nc.sync.dma_start(wv_sb[:], wv_d.rearrange("(o p) f -> p o f", p=128))
            nc.scalar.dma_start(cs_sb[:], cs_d[:])
            nc.scalar.dma_start(sn_sb[:], sn_d[:])
            nc.sync.dma_start(tri_sb[:], tri_d[:])
            nc.sync.dma_start(sel_sb[:], sel_d[:])
            c1 = slice(CHUNK, 2 * CHUNK)
            nc.sync.dma_start(xT_sb[:, :, c1], xT_r[:, :, c1])
            # xT chunks 2 and 3 are issued inside the pipeline loop so the
            # sync queue serves chunk-0/1 relayouts first

            # ones column for the Z denominators
            v4 = v_sb.rearrange("p st (h w) -> p st h w", h=HG)
            nc.gpsimd.memset(v4[:, :, :, 64:65], 1.0)


            def emit_qkproj(c, f0, dst_r, big):
                """projection of q (f0=0) or k (f0=256) for chunk c + rope.

                PSUM-reading multiplies run on DVE (Pool cannot read PSUM);
                the SBUF-only combines run on Pool."""
                csl = slice(c * CHUNK, (c + 1) * CHUNK)
                pq = pp.tile([128, 2, CHUNK], f32, tag=big, name=f"pq{big}")
                for e in range(ESUB):
                    kw = dict(start=(e == 0), stop=(e == ESUB - 1))
                    xs = xT_sb[:, e, csl]
                    nc.tensor.matmul(pq[:, 0, :], wqk_sb[:, e, f0:f0 + 128], xs, **kw)
                    nc.tensor.matmul(pq[:, 1, :], wqk_sb[:, e, f0 + 128:f0 + 256], xs, **kw)
                t1 = rsc.tile([128, CHUNK], f32, tag=f"t1{f0}", name="t1")
                t2 = rsc.tile([128, CHUNK], f32, tag=f"t2{f0}", name="t2")
                nc.vector.tensor_tensor(t1[:], pq[:, 0, :], cs_sb[:, csl], Alu.mult)
                nc.vector.tensor_tensor(t2[:], pq[:, 1, :], sn_sb[:, csl], Alu.mult)
                nc.gpsimd.tensor_tensor(dst_r[:, 0, csl], t1[:], t2[:], Alu.subtract)
                t3 = rsc.tile([128, CHUNK], f32, tag=f"t1{f0}", name="t3")
                t4 = rsc.tile([128, CHUNK], f32, tag=f"t2{f0}", name="t4")
                nc.vector.tensor_tensor(t3[:], pq[:, 0, :], sn_sb[:, csl], Alu.mult)
                nc.vector.tensor_tensor(t4[:], pq[:, 1, :], cs_sb[:, csl], Alu.mult)
                nc.gpsimd.tensor_tensor(dst_r[:, 1, csl], t3[:], t4[:], Alu.add)

            def emit_relayout(c, dst_r, dst_p):
                # pair layout: dst_p[:, p, csl] gets head 2p (parts 0:64) and
                # head 2p+1 (parts 64:128), x1/x2 interleaved per dim (a
                # shared q/k permutation of the contraction dim, so scores
                # are unchanged). One DMA per pair: the [64, 2, 512] ->
                # [128, 512] AP walk interleaves x1/x2 rows.
                csl = slice(c * CHUNK, (c + 1) * CHUNK)
                for p in range(2):
                    nc.sync.dma_start(dst_p[:, p, csl],
                                      dst_r[64 * p:64 * p + 64, :, csl])

            def emit_vproj(c):
                for st in range(4 * c, 4 * c + 4):
                    ssl = slice(st * 128, (st + 1) * 128)
                    pv = pp.tile([128, F_V], f32, name="pv",
                                 tag="pvz" if st % 2 == 0 else "po")
                    for e in range(ESUB):
                        nc.tensor.matmul(pv[:], xT_sb[:, e, ssl], wv_sb[:, e, :],
                                         start=(e == 0), stop=(e == ESUB - 1))
                    nc.vector.tensor_copy(
                        v_sb[:, st, :].rearrange("p (h w) -> p h w", h=HG)[:, :, 0:64],
                        pv[:].rearrange("p (h w) -> p h w", h=HG))

            def emit_attn(c, p, fill=None, fstate=None):
                i0 = c * BLK
                njt = 4 * c + 4
                ctx = [pp.tile([65, BLK], f32, tag=f"ctx{a}", name=f"ctx{a}")
                       for a in range(2)]
                for jt in range(njt):
                    r = jt - 4 * c
                    off = 128 * max(r, 0)
                    # interleave prev-chunk outproj units as PE filler so PE
                    # stays ahead of the (balanced) ACT exp stream
                    if fill:
                        fstate["done"] += 1
                        # quadratic ramp: back-load filler to where the ACT
                        # exp stream lags PE the most (late j-tiles); hold 3
                        # units back to cover the post-attention ACT drain
                        tgt = min((fstate["done"] ** 2 * fstate["units"]
                                   ) // fstate["jts"] ** 2,
                                  fstate["units"] - 3)
                        while fill and fstate["emitted"] < tgt:
                            fill.pop(0)()
                            fstate["emitted"] += 1
                    ps = pp.tile([128, 2, BLK], f32, tag=f"big{jt % 2}",
                                 name="ps")
                    for a in range(2):
                        nc.tensor.matmul(
                            ps[:, a, off:],
                            kp[64 * a:64 * a + 64, p, 128 * jt:128 * jt + 128],
                            qp[64 * a:64 * a + 64, p, i0 + off:i0 + BLK],
                            start=True, stop=True)
                    pt = ptp.tile([128, 2, BLK], f16, tag="pt", name="pt")
                    nc.scalar.activation(pt[:, :, off:], ps[:, :, off:],
                                         Act.Exp, scale=scale)
                    if r >= 0:
                        nc.gpsimd.tensor_tensor(
                            pt[:, :, off:off + 128],
                            pt[:, :, off:off + 128],
                            tri_sb[:, None, :].to_broadcast((128, 2, 128)),
                            Alu.mult)
                    # per-element has_written handles the ragged causal
                    # ranges; the 2KB-granular group check cannot
                    for a in range(2):
                        nc.tensor.matmul(
                            ctx[a][:, off:],
                            v_sb[:, jt, 65 * (2 * p + a):65 * (2 * p + a) + 65],
                            pt[:, a, off:],
                            start=(jt == 0), stop=(jt == njt - 1),
                            skip_group_check=True)
                # stash unnormalized ctx + Z rows
                for a in range(2):
                    nc.vector.tensor_copy(
                        ctxu_sb[64 * a:64 * a + 64, p, i0:i0 + BLK],
                        ctx[a][0:64, :])
                    zr = ptp.tile([1, BLK], f16, tag=f"zr{a}", name="zr",
                                  bufs=2)
                    nc.vector.tensor_copy(zr[:], ctx[a][64:65, :])
                    nc.sync.dma_start(zalls[p][a:a + 1, i0:i0 + BLK], zr[:])
                # per-pair softmax reciprocal: exp(-ln Z) on ACT (both in
                # the single natural_log_exp_and_others table set)
                lnz = smp.tile([2, BLK], f32, tag=f"lnz{p}", name="lnz")
                nc.scalar.activation(lnz[:], zalls[p][:, i0:i0 + BLK], Act.Ln)
                rz = smp.tile([2, BLK], f16, tag=f"rz{p}", name="rz")
                nc.scalar.activation(rz[:], lnz[:], Act.Exp, scale=-1.0)
                return rz

            def emit_norm(c, rzs):
                qsl = slice(c * BLK, (c + 1) * BLK)
                for p in range(2):
                    # zt lives in the (now idle) ctx banks so pv/po stay
                    # free of norm WARs at the chunk boundary
                    zt = pp.tile([128, BLK], f32, name="zt",
                                 tag="ctx0" if p == 0 else "ctx1")
                    nc.tensor.matmul(zt[:], sel_sb[:, 128 * p:128 * p + 128],
                                     rzs[p][:], start=True, stop=True)
                    nc.vector.tensor_tensor(ctxu_sb[:, p, qsl],
                                            ctxu_sb[:, p, qsl],
                                            zt[:], Alu.mult)

            def outproj_unit(st, n, tail=False):
                ssl = slice(st * 128, (st + 1) * 128)
                nsl = slice(n * 512, (n + 1) * 512)
                # double-buffer across the po and pvz banks
                po = pp.tile([128, 512], f32, name="po",
                             tag="po" if (2 * st + n) % 2 == 0 else "pvz")
                nc.tensor.matmul(po[:], ctxu_sb[:, 0, ssl],
                                 wout_sb[:, 0, nsl], start=True, stop=False)
                nc.tensor.matmul(po[:], ctxu_sb[:, 1, ssl],
                                 wout_sb[:, 1, nsl], start=False, stop=True)
                ot = otp.tile([128, 512], f16, tag="ot", name="ot")
                if tail and (2 * st + n) % 2 == 1:
                    # in the drain phase ACT and its DMA queue are idle:
                    # split the casts/stores across both engine paths
                    nc.scalar.copy(ot[:], po[:])
                    nc.scalar.dma_start(out_d[ssl, nsl], ot[:])
                else:
                    nc.vector.tensor_copy(ot[:], po[:])
                    nc.sync.dma_start(out_d[ssl, nsl], ot[:])

            def outproj_units(c):
                return [lambda st=st, n=n: outproj_unit(st, n)
                        for st in range(4 * c, 4 * c + 4) for n in range(2)]

            def emit_outproj(c, tail=False):
                for st in range(4 * c, 4 * c + 4):
                    for n in range(2):
                        outproj_unit(st, n, tail=tail)

            # ---- fused per-chunk pipeline ----
            for c in range(NCHUNK):
                # Q then K then V: the rope->relayout chains for q and k
                # complete during V-proj, so attention never waits on them
                emit_qkproj(c, 0, qr, "big0")
                emit_relayout(c, qr, qp)
                emit_qkproj(c, 256, kr, "big1")
                emit_relayout(c, kr, kp)
                emit_vproj(c)
                if c == 0:
                    nc.scalar.dma_start(
                        wout_sb[:], wout_d.rearrange("(o p) e -> p o e", p=128))
                fill = outproj_units(c - 1) if c >= 1 else []
                fstate = {"done": 0, "emitted": 0, "units": len(fill),
                          "jts": 2 * (4 * c + 4)}
                rz0 = emit_attn(c, 0, fill, fstate)
                if c + 2 < NCHUNK:
                    cn = slice((c + 2) * CHUNK, (c + 3) * CHUNK)
                    nc.sync.dma_start(xT_sb[:, :, cn], xT_r[:, :, cn])
                rz1 = emit_attn(c, 1, fill, fstate)
                for u in fill:
                    u()
                emit_norm(c, (rz0, rz1))
            emit_outproj(NCHUNK - 1, tail=True)

    nc.compile()
    return nc


def _host_inputs(x, W_qkv, W_out):
    """Build the 8 per-core input maps."""
    x = np.asarray(x, dtype=np.float32)
    W_qkv = np.asarray(W_qkv, dtype=np.float32)
    W_out = np.asarray(W_out, dtype=np.float32)

    pos = np.arange(S)
    freqs = 1.0 / 10000.0 ** (np.arange(0, HEAD_DIM, 2) / HEAD_DIM)
    ang = pos[:, None] * freqs[None, :]            # (S, 32)
    cs32 = np.cos(ang).T.astype(np.float16)        # (32, S)
    sn32 = np.sin(ang).T.astype(np.float16)
    cs = np.tile(cs32, (4, 1))                     # (128, S)
    sn = np.tile(sn32, (4, 1))
    tri = (np.arange(128)[:, None] <= np.arange(128)[None, :]).astype(np.float16)
    # selector for Z broadcast: sel[k, 128p+m] = 1 where k == m//64
    # (per-pair: the moving rz holds that pair's two head rows)
    sel = np.zeros((2, 256), np.float16)
    for p in range(2):
        for m in range(128):
            sel[m // 64, 128 * p + m] = 1.0

    in_maps = []
    for b in range(B):
        xT = np.ascontiguousarray(x[b].T.astype(np.float16))
        for g in range(NG):
            heads = np.arange(HG * g, HG * g + HG)
            qa = np.concatenate([0 * NUM_HEADS * HEAD_DIM + h * HEAD_DIM
                                 + np.arange(0, HEAD_DIM, 2) for h in heads])
            qb = qa + 1
            ka = qa + NUM_HEADS * HEAD_DIM
            kb = ka + 1
            wqk = np.ascontiguousarray(
                W_qkv[:, np.concatenate([qa, qb, ka, kb])].astype(np.float16))
            vcols = np.concatenate([2 * NUM_HEADS * HEAD_DIM + h * HEAD_DIM
                                    + np.arange(HEAD_DIM) for h in heads])
            wv = np.ascontiguousarray(W_qkv[:, vcols].astype(np.float16))
            wout = np.ascontiguousarray(
                W_out[HG * g * HEAD_DIM:HG * (g + 1) * HEAD_DIM].astype(np.float16))
            in_maps.append({"xT": xT, "wqk": wqk, "wv": wv, "wout": wout,
                            "cs": cs, "sn": sn, "tri": tri, "sel": sel})
    return in_maps


def get_program():
    if "nc" not in _CACHE:
        _CACHE["nc"] = _build_program()
    return _CACHE["nc"]


def run(x, W_qkv, W_out, trace=False, tmpdir=None):
    from concourse import bass_utils
    nc = get_program()
    in_maps = _host_inputs(x, W_qkv, W_out)
    res = bass_utils.run_bass_kernel_spmd(
        nc, in_maps, core_ids=list(range(N_CORES)), trace=trace, tmpdir=tmpdir)
    out = np.zeros((B, S, E), np.float32)
    for b in range(B):
        for g in range(NG):
            out[b] += res.results[b * NG + g]["out"].astype(np.float32)
    return out, res


def kernel(x, W_qkv, W_out):
    out, _ = run(x, W_qkv, W_out)
    return out


# revision 53
# speedup vs baseline: 1.0229x; 1.0229x over previous
"""Causal self-attention (RoPE) Trainium2 kernel, 8-way sharded.

Sharding: core = (batch b in 0..1) x (head group g in 0..3, 4 heads each).
Each core computes its batch's attention for its 4 heads plus the partial
output projection; the host sums the 4 partials per batch.

v2: single fused per-chunk pipeline. Each 512-seq chunk does
QKV projection -> rope -> attention block (chunk i-block) -> per-block
normalization -> (next iter) output projection, so PE stays continuously
busy, DMA/ACT/DVE/Pool overlap, and output DMA streams per block.

Layout strategy (per core):
- host passes xT = x[b].T (fp16) so the embed dim lands on SBUF partitions.
- W_qkv columns are permuted so q^T/k^T emerge from the projection matmul
  already transposed, with RoPE even/odd dim pairs de-interleaved into
  x1/x2 partition blocks (scores are invariant to a head-dim permutation).
- all matmul operands are fp16; PSUM accumulation stays fp32.
- scores are computed transposed (sT[j,i]); softmax needs no max pass
  (|scores| < ~4) and the denominator is obtained by appending a ones
  column to V (M=65 PV matmuls). Normalization multiplies by exp(-ln Z)
  broadcast via a selector matmul, per i-block.
- causal masking: only j<=i column ranges are computed; the single
  diagonal 128x128 block per j-tile gets a triangle multiply.
- engine split: exp on ACT; rope-q/stash/casts on DVE; rope-k/triangle/
  norm-mult/casts on Pool; scores/PV/projections on PE.
"""
import sys

sys.path.insert(0, "/opt/trn_rl_repo")

import numpy as np

NUM_HEADS = 16
HEAD_DIM = 64
B, S, E = 2, 2048, 1024
HG = 4                      # heads per core
NG = NUM_HEADS // HG        # head groups
N_CORES = B * NG
F_QK = 2 * HG * HEAD_DIM    # 512 projected q+k rows per core
F_V = HG * HEAD_DIM         # 256 v cols per core
ESUB = E // 128             # 8 K-subtiles over embed dim
NCHUNK = 4                  # 512-col seq chunks (projection + i-blocks)
CHUNK = S // NCHUNK         # 512
NST = S // 128              # 16 seq tiles of 128
BLK = 512                   # attention i-block width == CHUNK
NBLK = S // BLK             # 4

_CACHE = {}


def _build_program():
    import concourse.bass as bass
    import concourse.mybir as mybir
    import concourse.tile as tile
    from concourse import bacc

    f32 = mybir.dt.float32
    f16 = mybir.dt.float16
    Alu = mybir.AluOpType
    Act = mybir.ActivationFunctionType

    # Force the single combined Ln+Exp activation-table set: hide Exp/Ln in
    # every other set so the table-load fixpoint can't alternate between
    # exp-only and ln-only sets (each mid-stream ACT_TABLE_LOAD costs ~1.3us
    # on the exp-critical scalar engine).
    from concourse.hw_specs import get_activation_tables as _gat
    if not getattr(bacc, "_one_set_patch", False):
        def _tables_one_set(arch, __orig=_gat):
            out = {}
            for name, funcs in __orig(arch).items():
                if name == "natural_log_exp_and_others":
                    out[name] = funcs
                else:
                    out[name] = {f for f in funcs
                                 if f not in (Act.Exp, Act.Ln)}
            return out
        bacc.get_activation_tables = _tables_one_set
        bacc._one_set_patch = True

    nc = bacc.Bacc("TRN2", target_bir_lowering=False, debug=False,
                   num_devices=N_CORES)

    xT_d = nc.dram_tensor("xT", (E, S), f16, kind="ExternalInput").ap()
    wqk_d = nc.dram_tensor("wqk", (E, F_QK), f16, kind="ExternalInput").ap()
    wv_d = nc.dram_tensor("wv", (E, F_V), f16, kind="ExternalInput").ap()
    wout_d = nc.dram_tensor("wout", (F_V, E), f16, kind="ExternalInput").ap()
    cs_d = nc.dram_tensor("cs", (128, S), f16, kind="ExternalInput").ap()
    sn_d = nc.dram_tensor("sn", (128, S), f16, kind="ExternalInput").ap()
    tri_d = nc.dram_tensor("tri", (128, 128), f16, kind="ExternalInput").ap()
    sel_d = nc.dram_tensor("sel", (2, 256), f16, kind="ExternalInput").ap()
    out_d = nc.dram_tensor("out", (S, E), f16, kind="ExternalOutput").ap()

    scale = 1.0 / float(np.sqrt(HEAD_DIM))

    with tile.TileContext(nc) as tc:
        with tc.tile_pool(name="per", bufs=1) as per, \
             tc.tile_pool(name="rsc", bufs=2) as rsc, \
             tc.tile_pool(name="pt", bufs=3) as ptp, \
             tc.tile_pool(name="ot", bufs=4) as otp, \
             tc.tile_pool(name="sm", bufs=2) as smp, \
             tc.tile_pool(name="pp", bufs=1, space="PSUM") as pp:
            # ---- persistent SBUF tiles ----
            xT_sb = per.tile([128, ESUB, S], f16)
            wqk_sb = per.tile([128, ESUB, F_QK], f16)
            wv_sb = per.tile([128, ESUB, F_V], f16)
            wout_sb = per.tile([128, 2, E], f16)
            cs_sb = per.tile([128, S], f16)
            sn_sb = per.tile([128, S], f16)
            tri_sb = per.tile([128, 128], f16)
            sel_sb = per.tile([2, 256], f16)
            qr = per.tile([128, 2, S], f16)    # rope out q: [:,0]=x1', [:,1]=x2'
            kr = per.tile([128, 2, S], f16)
            qp = per.tile([128, 2, S], f16)    # pair layout for scores
            kp = per.tile([128, 2, S], f16)
            v_sb = per.tile([128, NST, HG * 65], f16)
            ctxu_sb = per.tile([128, 2, S], f16)
            zall0 = per.tile([2, S], f16)
            zall1 = per.tile([2, S], f16)
            zalls = (zall0, zall1)

            # ---- input DMAs, split across the two HWDGE queues (sync=SP,
            # scalar=ACT) ordered by first use; first-needed tensors are
            # split in half so compute can start sooner ----
            xT_r = xT_d.rearrange("(o p) s -> p o s", p=128)
            wqk_r = wqk_d.rearrange("(o p) f -> p o f", p=128)
            c0 = slice(0, CHUNK)
            nc.sync.dma_start(xT_sb[:, 0:4, c0], xT_r[:, 0:4, c0])
            nc.scalar.dma_start(wqk_sb[:, 0:4, :], wqk_r[:, 0:4, :])
            nc.scalar.dma_start(wqk_sb[:, 4:8, :], wqk_r[:, 4:8, :])
            nc.scalar.dma_start(xT_sb[:, 4:8, c0], xT_r[:, 4:8, c0])
            nc.sync.dma_start(wv_sb[:], wv_d.rearrange("(o p) f -> p o f", p=128))
            nc.scalar.dma_start(cs_sb[:], cs_d[:])
            nc.scalar.dma_start(sn_sb[:], sn_d[:])
            nc.sync.dma_start(tri_sb[:], tri_d[:])
            nc.sync.dma_start(sel_sb[:], sel_d[:])
            c1 = slice(CHUNK, 2 * CHUNK)
            nc.sync.dma_start(xT_sb[:, :, c1], xT_r[:, :, c1])
            # xT chunks 2 and 3 are issued inside the pipeline loop so the
            # sync queue serves chunk-0/1 relayouts first

            # ones column for the Z denominators
            v4 = v_sb.rearrange("p st (h w) -> p st h w", h=HG)
            nc.gpsimd.memset(v4[:, :, :, 64:65], 1.0)


            def emit_qkproj(c, f0, dst_r, big):
                """projection of q (f0=0) or k (f0=256) for chunk c + rope.

                PSUM-reading multiplies run on DVE (Pool cannot read PSUM);
                the SBUF-only combines run on Pool."""
                csl = slice(c * CHUNK, (c + 1) * CHUNK)
                pq = pp.tile([128, 2, CHUNK], f32, tag=big, name=f"pq{big}")
                for e in range(ESUB):
                    kw = dict(start=(e == 0), stop=(e == ESUB - 1))
                    xs = xT_sb[:, e, csl]
                    nc.tensor.matmul(pq[:, 0, :], wqk_sb[:, e, f0:f0 + 128], xs, **kw)
                    nc.tensor.matmul(pq[:, 1, :], wqk_sb[:, e, f0 + 128:f0 + 256], xs, **kw)
                t1 = rsc.tile([128, CHUNK], f32, tag=f"t1{f0}", name="t1")
                t2 = rsc.tile([128, CHUNK], f32, tag=f"t2{f0}", name="t2")
                nc.vector.tensor_tensor(t1[:], pq[:, 0, :], cs_sb[:, csl], Alu.mult)
                nc.vector.tensor_tensor(t2[:], pq[:, 1, :], sn_sb[:, csl], Alu.mult)
                nc.gpsimd.tensor_tensor(dst_r[:, 0, csl], t1[:], t2[:], Alu.subtract)
                t3 = rsc.tile([128, CHUNK], f32, tag=f"t1{f0}", name="t3")
                t4 = rsc.tile([128, CHUNK], f32, tag=f"t2{f0}", name="t4")
                nc.vector.tensor_tensor(t3[:], pq[:, 0, :], sn_sb[:, csl], Alu.mult)
                nc.vector.tensor_tensor(t4[:], pq[:, 1, :], cs_sb[:, csl], Alu.mult)
                nc.gpsimd.tensor_tensor(dst_r[:, 1, csl], t3[:], t4[:], Alu.add)

            def emit_relayout(c, dst_r, dst_p):
                # pair layout: dst_p[:, p, csl] gets head 2p (parts 0:64) and
                # head 2p+1 (parts 64:128), x1/x2 interleaved per dim (a
                # shared q/k permutation of the contraction dim, so scores
                # are unchanged). One DMA per pair: the [64, 2, 512] ->
                # [128, 512] AP walk interleaves x1/x2 rows.
                csl = slice(c * CHUNK, (c + 1) * CHUNK)
                for p in range(2):
                    nc.sync.dma_start(dst_p[:, p, csl],
                                      dst_r[64 * p:64 * p + 64, :, csl])

            def emit_vproj(c):
                for st in range(4 * c, 4 * c + 4):
                    ssl = slice(st * 128, (st + 1) * 128)
                    pv = pp.tile([128, F_V], f32, name="pv",
                                 tag="pvz" if st % 2 == 0 else "po")
                    for e in range(ESUB):
                        nc.tensor.matmul(pv[:], xT_sb[:, e, ssl], wv_sb[:, e, :],
                                         start=(e == 0), stop=(e == ESUB - 1))
                    nc.vector.tensor_copy(
                        v_sb[:, st, :].rearrange("p (h w) -> p h w", h=HG)[:, :, 0:64],
                        pv[:].rearrange("p (h w) -> p h w", h=HG))

            def emit_attn(c, p, fill=None, fstate=None):
                i0 = c * BLK
                njt = 4 * c + 4
                ctx = [pp.tile([65, BLK], f32, tag=f"ctx{a}", name=f"ctx{a}")
                       for a in range(2)]
                for jt in range(njt):
                    r = jt - 4 * c
                    off = 128 * max(r, 0)
                    # interleave prev-chunk outproj units as PE filler so PE
                    # stays ahead of the (balanced) ACT exp stream
                    if fill:
                        fstate["done"] += 1
                        # quadratic ramp: back-load filler to where the ACT
                        # exp stream lags PE the most (late j-tiles)
                        tgt = (fstate["done"] ** 2 * fstate["units"]
                               ) // fstate["jts"] ** 2
                        while fill and fstate["emitted"] < tgt:
                            fill.pop(0)()
                            fstate["emitted"] += 1
                    ps = pp.tile([128, 2, BLK], f32, tag=f"big{jt % 2}",
                                 name="ps")
                    for a in range(2):
                        nc.tensor.matmul(
                            ps[:, a, off:],
                            kp[64 * a:64 * a + 64, p, 128 * jt:128 * jt + 128],
                            qp[64 * a:64 * a + 64, p, i0 + off:i0 + BLK],
                            start=True, stop=True)
                    pt = ptp.tile([128, 2, BLK], f16, tag="pt", name="pt")
                    nc.scalar.activation(pt[:, :, off:], ps[:, :, off:],
                                         Act.Exp, scale=scale)
                    if r >= 0:
                        nc.gpsimd.tensor_tensor(
                            pt[:, :, off:off + 128],
                            pt[:, :, off:off + 128],
                            tri_sb[:, None, :].to_broadcast((128, 2, 128)),
                            Alu.mult)
                    # per-element has_written handles the ragged causal
                    # ranges; the 2KB-granular group check cannot
                    for a in range(2):
                        nc.tensor.matmul(
                            ctx[a][:, off:],
                            v_sb[:, jt, 65 * (2 * p + a):65 * (2 * p + a) + 65],
                            pt[:, a, off:],
                            start=(jt == 0), stop=(jt == njt - 1),
                            skip_group_check=True)
                # stash unnormalized ctx + Z rows
                for a in range(2):
                    nc.vector.tensor_copy(
                        ctxu_sb[64 * a:64 * a + 64, p, i0:i0 + BLK],
                        ctx[a][0:64, :])
                    zr = ptp.tile([1, BLK], f16, tag=f"zr{a}", name="zr",
                                  bufs=2)
                    nc.vector.tensor_copy(zr[:], ctx[a][64:65, :])
                    nc.sync.dma_start(zalls[p][a:a + 1, i0:i0 + BLK], zr[:])
                # per-pair softmax reciprocal: exp(-ln Z) on ACT (both in
                # the single natural_log_exp_and_others table set)
                lnz = smp.tile([2, BLK], f32, tag=f"lnz{p}", name="lnz")
                nc.scalar.activation(lnz[:], zalls[p][:, i0:i0 + BLK], Act.Ln)
                rz = smp.tile([2, BLK], f16, tag=f"rz{p}", name="rz")
                nc.scalar.activation(rz[:], lnz[:], Act.Exp, scale=-1.0)
                return rz

            def emit_norm(c, rzs):
                qsl = slice(c * BLK, (c + 1) * BLK)
                for p in range(2):
                    zt = pp.tile([128, BLK], f32, name="zt",
                                 tag="pvz" if p == 0 else "po")
                    nc.tensor.matmul(zt[:], sel_sb[:, 128 * p:128 * p + 128],
                                     rzs[p][:], start=True, stop=True)
                    nc.vector.tensor_tensor(ctxu_sb[:, p, qsl],
                                            ctxu_sb[:, p, qsl],
                                            zt[:], Alu.mult)

            def outproj_unit(st, n, tail=False):
                ssl = slice(st * 128, (st + 1) * 128)
                nsl = slice(n * 512, (n + 1) * 512)
                # double-buffer across the po and pvz banks
                po = pp.tile([128, 512], f32, name="po",
                             tag="po" if (2 * st + n) % 2 == 0 else "pvz")
                nc.tensor.matmul(po[:], ctxu_sb[:, 0, ssl],
                                 wout_sb[:, 0, nsl], start=True, stop=False)
                nc.tensor.matmul(po[:], ctxu_sb[:, 1, ssl],
                                 wout_sb[:, 1, nsl], start=False, stop=True)
                ot = otp.tile([128, 512], f16, tag="ot", name="ot")
                if tail and (2 * st + n) % 2 == 1:
                    # in the drain phase ACT and its DMA queue are idle:
                    # split the casts/stores across both engine paths
                    nc.scalar.copy(ot[:], po[:])
                    nc.scalar.dma_start(out_d[ssl, nsl], ot[:])
                else:
                    nc.vector.tensor_copy(ot[:], po[:])
                    nc.sync.dma_start(out_d[ssl, nsl], ot[:])

            def outproj_units(c):
                return [lambda st=st, n=n: outproj_unit(st, n)
                        for st in range(4 * c, 4 * c + 4) for n in range(2)]

            def emit_outproj(c, tail=False):
                for st in range(4 * c, 4 * c + 4):
                    for n in range(2):
                        outproj_unit(st, n, tail=tail)

            # ---- fused per-chunk pipeline ----
            for c in range(NCHUNK):
                # Q then K then V: the rope->relayout chains for q and k
                # complete during V-proj, so attention never waits on them
                emit_qkproj(c, 0, qr, "big0")
                emit_relayout(c, qr, qp)
                emit_qkproj(c, 256, kr, "big1")
                emit_relayout(c, kr, kp)
                emit_vproj(c)
                if c == 0:
                    nc.scalar.dma_start(
                        wout_sb[:], wout_d.rearrange("(o p) e -> p o e", p=128))
                fill = outproj_units(c - 1) if c >= 1 else []
                fstate = {"done": 0, "emitted": 0, "units": len(fill),
                          "jts": 2 * (4 * c + 4)}
                rz0 = emit_attn(c, 0, fill, fstate)
                if c + 2 < NCHUNK:
                    cn = slice((c + 2) * CHUNK, (c + 3) * CHUNK)
                    nc.sync.dma_start(xT_sb[:, :, cn], xT_r[:, :, cn])
                rz1 = emit_attn(c, 1, fill, fstate)
                for u in fill:
                    u()
                emit_norm(c, (rz0, rz1))
            emit_outproj(NCHUNK - 1, tail=True)

    nc.compile()
    return nc


def _host_inputs(x, W_qkv, W_out):
    """Build the 8 per-core input maps."""
    x = np.asarray(x, dtype=np.float32)
    W_qkv = np.asarray(W_qkv, dtype=np.float32)
    W_out = np.asarray(W_out, dtype=np.float32)

    pos = np.arange(S)
    freqs = 1.0 / 10000.0 ** (np.arange(0, HEAD_DIM, 2) / HEAD_DIM)
    ang = pos[:, None] * freqs[None, :]            # (S, 32)
    cs32 = np.cos(ang).T.astype(np.float16)        # (32, S)
    sn32 = np.sin(ang).T.astype(np.float16)
    cs = np.tile(cs32, (4, 1))                     # (128, S)
    sn = np.tile(sn32, (4, 1))
    tri = (np.arange(128)[:, None] <= np.arange(128)[None, :]).astype(np.float16)
    # selector for Z broadcast: sel[k, 128p+m] = 1 where k == m//64
    # (per-pair: the moving rz holds that pair's two head rows)
    sel = np.zeros((2, 256), np.float16)
    for p in range(2):
        for m in range(128):
            sel[m // 64, 128 * p + m] = 1.0

    in_maps = []
    for b in range(B):
        xT = np.ascontiguousarray(x[b].T.astype(np.float16))
        for g in range(NG):
            heads = np.arange(HG * g, HG * g + HG)
            qa = np.concatenate([0 * NUM_HEADS * HEAD_DIM + h * HEAD_DIM
                                 + np.arange(0, HEAD_DIM, 2) for h in heads])
            qb = qa + 1
            ka = qa + NUM_HEADS * HEAD_DIM
            kb = ka + 1
            wqk = np.ascontiguousarray(
                W_qkv[:, np.concatenate([qa, qb, ka, kb])].astype(np.float16))
            vcols = np.concatenate([2 * NUM_HEADS * HEAD_DIM + h * HEAD_DIM
                                    + np.arange(HEAD_DIM) for h in heads])
            wv = np.ascontiguousarray(W_qkv[:, vcols].astype(np.float16))
            wout = np.ascontiguousarray(
                W_out[HG * g * HEAD_DIM:HG * (g + 1) * HEAD_DIM].astype(np.float16))
            in_maps.append({"xT": xT, "wqk": wqk, "wv": wv, "wout": wout,
                            "cs": cs, "sn": sn, "tri": tri, "sel": sel})
    return in_maps


def get_program():
    if "nc" not in _CACHE:
        _CACHE["nc"] = _build_program()
    return _CACHE["nc"]


def run(x, W_qkv, W_out, trace=False, tmpdir=None):
    from concourse import bass_utils
    nc = get_program()
    in_maps = _host_inputs(x, W_qkv, W_out)
    res = bass_utils.run_bass_kernel_spmd(
        nc, in_maps, core_ids=list(range(N_CORES)), trace=trace, tmpdir=tmpdir)
    out = np.zeros((B, S, E), np.float32)
    for b in range(B):
        for g in range(NG):
            out[b] += res.results[b * NG + g]["out"].astype(np.float32)
    return out, res


def kernel(x, W_qkv, W_out):
    out, _ = run(x, W_qkv, W_out)
    return out
